# revision 8
# baseline (speedup 1.0000x reference)
"""Graph Wavelet NN (2-layer) Trainium2 kernel, 8-core row-parallel, v2.

Math per layer: out = (wavelets * f) @ (wavelets_inv @ (x @ W)); the filter is
folded into a row-scale of the small spectral tensor s.

v2 design (v1 measured ~477us: cc_op_active 253us, PE cold-throttled 35%,
collectives serialized after each compute phase):
- t1 = x @ W1 is computed FULLY REPLICATED on every core (27us of PE work
  with zero cross-core deps).  This absorbs the one-time ~50us collective
  stream init and lets s1 run with all inputs SBUF-resident - layer 1 has no
  input exchange at all.
- winvT is SBUF-RESIDENT (16MB, filled during s1, reused by s2); wT is
  streamed twice (during o1 and o2).  Each phase window streams <=16MB, so
  HBM never has two big streams stacked.
- s1 is computed in ROW-QUARTER passes, each quarter stored + AllGather'd
  immediately -> the s1 exchange pipelines against s1's remaining passes.
  s2/o1/o2 are slot-pipelined single passes (max tolerance to AG arrival),
  consuming local data first, then remote granules in arrival order.
- One host-side permutation pi = [own 8 blocks | quarter-major, cyclic-rank
  remote 56 blocks] orders the contraction dim of winvT and wT so program
  order == arrival order.  Collectives alone on gpsimd; winv/wT streams on
  sync; small loads/stores/gather reads on scalar.  bf16 MMs, fp32 PSUM.
"""

import sys

if "/opt/trn_rl_repo" not in sys.path:
    sys.path.insert(0, "/opt/trn_rl_repo")

import numpy as np
import ml_dtypes

import concourse.bass as bass
import concourse.mybir as mybir
import concourse.tile as tile
from concourse import bacc, bass_utils

N = 8192
F = 512
C = 256
NCORES = 8
R = N // NCORES          # 1024 rows per core
Q = R // 4               # 256-row quarter
NREM = NCORES - 1        # 7 remote ranks
NSLOT = N // 128         # 64 contraction slots of 128 rows

F32 = mybir.dt.float32
BF16 = mybir.dt.bfloat16
NP_BF16 = ml_dtypes.bfloat16


def build_kernel(sim_single_core=False):
    nc = bacc.Bacc(
        "TRN2",
        target_bir_lowering=False,
        debug=False,
        num_devices=1 if sim_single_core else NCORES,
    )

    xT = nc.dram_tensor("xT", [F, N], BF16, kind="ExternalInput")
    w1 = nc.dram_tensor("w1", [F, C], BF16, kind="ExternalInput")
    w2 = nc.dram_tensor("w2", [C, C], BF16, kind="ExternalInput")
    winvT = nc.dram_tensor("winvT", [N, R], BF16, kind="ExternalInput")
    wT = nc.dram_tensor("wT", [N, R], BF16, kind="ExternalInput")
    f1 = nc.dram_tensor("f1", [R], F32, kind="ExternalInput")
    f2 = nc.dram_tensor("f2", [R], F32, kind="ExternalInput")
    outT = nc.dram_tensor("outT", [C, R], F32, kind="ExternalOutput")

    rg = [list(range(NCORES))]

    with tile.TileContext(nc) as tc:
        with (
            tc.tile_pool(name="dram", bufs=1, space="DRAM") as dram,
            tc.tile_pool(name="const", bufs=1) as const,
            tc.tile_pool(name="wtp", bufs=2) as wtp,
            tc.tile_pool(name="xqp", bufs=2) as xqp,
            tc.tile_pool(name="tsp", bufs=5) as tsp,
            tc.tile_pool(name="psq", bufs=2, space="PSUM") as psq,
            tc.tile_pool(name="psO", bufs=2, space="PSUM") as psO,
        ):
            # ---- DRAM exchange buffers (quarter granules everywhere) ----
            def mk_pair(nm):
                ins, outs = [], []
                for b in range(4):
                    ins.append(dram.tile([Q, C], BF16, name=f"{nm}{b}_d"))
                    outs.append(
                        dram.tile(
                            [NCORES * Q, C], BF16,
                            addr_space="Local" if sim_single_core else "Shared",
                            name=f"{nm}{b}g_d",
                        )
                    )
                return ins, outs

            s1q_d, s1g_d = mk_pair("s1")
            t2q_d, t2g_d = mk_pair("t2")
            s2q_d, s2g_d = mk_pair("s2")

            # ---- persistent SBUF ----
            winv_sb = const.tile([128, NSLOT, R], BF16)     # 128KB/part
            t1f_sb = const.tile([128, NSLOT, C], BF16)      # full t1, pi-order
            t_sb2 = const.tile([128, 8, C], BF16)           # local t2 rows
            s_sb1 = const.tile([128, 8, C], BF16)           # local s1 rows
            s_sb2 = const.tile([128, 8, C], BF16)           # local s2 rows
            h1T_sb = const.tile([128, C // 128, R], BF16)   # relu(o1).T
            w1_sb = const.tile([128, F // 128, C], BF16)
            w2_sb = const.tile([128, C // 128, C], BF16)
            f1_sb = const.tile([128, 8], F32)
            f2_sb = const.tile([128, 8], F32)

            nc.scalar.dma_start(
                out=w1_sb[:], in_=w1.ap().rearrange("(kc p) n -> p kc n", p=128)
            )
            nc.scalar.dma_start(
                out=w2_sb[:], in_=w2.ap().rearrange("(kc p) n -> p kc n", p=128)
            )
            nc.scalar.dma_start(
                out=f1_sb[:], in_=f1.ap().rearrange("(mt p) -> p mt", p=128)
            )
            nc.scalar.dma_start(
                out=f2_sb[:], in_=f2.ap().rearrange("(mt p) -> p mt", p=128)
            )

            def all_gather(in_d, out_d):
                if sim_single_core:
                    for rr in range(NCORES):
                        nc.sync.dma_start(
                            out=out_d[rr * Q:(rr + 1) * Q, :], in_=in_d[:, :]
                        )
                else:
                    nc.gpsimd.collective_compute(
                        "AllGather",
                        mybir.AluOpType.bypass,
                        replica_groups=rg,
                        ins=[in_d.opt()],
                        outs=[out_d.opt()],
                    )

            # cyclic remote-rank ids; host permutes wavelet rows to match
            pid = nc.scalar.partition_id()
            rrs = [(pid + (1 + s)) & 7 for s in range(NREM)]

            # winv fill: q-major so s1's quarter-0 pass streams just-in-time.
            for q in range(4):
                for g in range(8):
                    nc.sync.dma_start(
                        out=winv_sb[:, g * 8:(g + 1) * 8, q * Q:(q + 1) * Q],
                        in_=winvT.ap()[
                            g * 1024:(g + 1) * 1024, q * Q:(q + 1) * Q
                        ].rearrange("(kc p) m -> p kc m", p=128),
                    )

            # ======= t1 = x @ W1, fully replicated, staged in pi order =======
            # granule = 256 x-columns (2 slots); 32 granules cover all 8192.
            def t1_granule(gi, col0):
                xq = xqp.tile([128, 4, C], BF16, tag="xq", name=f"xq{gi}")
                nc.scalar.dma_start(
                    out=xq[:],
                    in_=xT.ap()[:, col0].rearrange("(kc p) m -> p kc m", p=128),
                )
                pt = psq.tile([128, 2, C], F32, tag="ps", name=f"pt1_{gi}")
                for j in range(2):
                    for kc in range(4):
                        nc.tensor.matmul(
                            pt[:, j, :],
                            xq[:, kc, j * 128:(j + 1) * 128],
                            w1_sb[:, kc, :],
                            start=(j == 0 and kc == 0),
                            stop=(kc == 3),
                            skip_group_check=True,
                        )
                    nc.vector.tensor_copy(t1f_sb[:, 2 * gi + j, :], pt[:, j, :])

            for g in range(4):  # local quarters -> slots 0..7
                t1_granule(g, bass.ds(pid * R + g * Q, Q))
            for r in range(28):  # remote quarters -> slots 8..63
                qq, s = divmod(r, NREM)
                t1_granule(4 + r, bass.ds(rrs[s] * R + qq * Q, Q))

            # ======= s1 = Winv @ t1 (all SBUF), row-quarter passes + AG =======
            for q in range(4):
                ps = psq.tile([128, 2, C], F32, tag="ps", name=f"ps1_{q}")
                for p in range(NSLOT):
                    for j in range(2):
                        nc.tensor.matmul(
                            ps[:, j, :],
                            winv_sb[:, p, q * Q + j * 128:q * Q + (j + 1) * 128],
                            t1f_sb[:, p, :],
                            start=(p == 0 and j == 0),
                            stop=(p == NSLOT - 1),
                            skip_group_check=True,
                        )
                for j in range(2):
                    nc.vector.tensor_scalar_mul(
                        s_sb1[:, 2 * q + j, :],
                        ps[:, j, :],
                        f1_sb[:, 2 * q + j:2 * q + j + 1],
                    )
                nc.scalar.dma_start(
                    out=s1q_d[q][:, :].rearrange("(k p) n -> p k n", p=128),
                    in_=s_sb1[:, 2 * q:2 * q + 2, :],
                )
                all_gather(s1q_d[q], s1g_d[q])

            # ---- o phase: out_loc = (w[rows]*f) @ s_full, slot-pipelined ----
            def o_phase(s_sb, sg_d, drain_cb, name):
                po = [
                    psO.tile([128, R], F32, tag="po", name=f"po_{name}{ch}")
                    for ch in range(2)
                ]
                wt_tiles = {}

                def load_wt(g):
                    t = wtp.tile([128, 4, R], BF16, tag="wt", name=f"wt_{name}{g}")
                    nc.sync.dma_start(
                        out=t[:],
                        in_=wT.ap()[g * 512:(g + 1) * 512, :].rearrange(
                            "(kc p) m -> p kc m", p=128
                        ),
                    )
                    wt_tiles[g] = t

                def mm(p, lhsT_of):
                    g = p // 4
                    for ch in range(2):
                        for mh in range(2):
                            nc.tensor.matmul(
                                po[ch][:, mh * 512:(mh + 1) * 512],
                                lhsT_of(ch),
                                wt_tiles[g][:, p % 4, mh * 512:(mh + 1) * 512],
                                start=(p == 0),
                                stop=(p == NSLOT - 1),
                                skip_group_check=True,
                            )

                load_wt(0)
                load_wt(1)
                # local slots 0..7
                for p in range(8):
                    mm(p, lambda ch, _p=p: s_sb[:, _p, ch * 128:(ch + 1) * 128])
                load_wt(2)
                load_wt(3)
                # remote granules, arrival order (quarter-major, cyclic rank)
                for r in range(28):
                    qq, s = divmod(r, NREM)
                    sgt = tsp.tile(
                        [128, 2, C], BF16, tag="ts", name=f"so_{name}_{r}"
                    )
                    nc.scalar.dma_start(
                        out=sgt[:],
                        in_=sg_d[qq][bass.ts(rrs[s], Q), :].rearrange(
                            "(k p) n -> p k n", p=128
                        ),
                    )
                    if r % 2 == 0 and 4 + r // 2 < 16:
                        load_wt(4 + r // 2)
                    for k in range(2):
                        p = 8 + 2 * r + k
                        mm(
                            p,
                            lambda ch, _t=sgt, _k=k: _t[
                                :, _k, ch * 128:(ch + 1) * 128
                            ],
                        )
                for ch in range(2):
                    drain_cb(ch, po[ch])

            # ================= layer 1 out =================
            def relu_drain(ch, po):
                for mh in range(2):
                    nc.vector.tensor_scalar_max(
                        h1T_sb[:, ch, mh * 512:(mh + 1) * 512],
                        po[:, mh * 512:(mh + 1) * 512],
                        0.0,
                    )

            o_phase(s_sb1, s1g_d, relu_drain, "o1")

            # ======= t2 = relu(o1) @ W2 (local rows), quartered + AG =======
            for q in range(4):
                pt = psq.tile([128, 2, C], F32, tag="ps", name=f"pt2_{q}")
                for j in range(2):
                    mt = 2 * q + j
                    for kc in range(2):
                        nc.tensor.matmul(
                            pt[:, j, :],
                            h1T_sb[:, kc, mt * 128:(mt + 1) * 128],
                            w2_sb[:, kc, :],
                            start=(j == 0 and kc == 0),
                            stop=(kc == 1),
                            skip_group_check=True,
                        )
                    nc.vector.tensor_copy(t_sb2[:, mt, :], pt[:, j, :])
                nc.scalar.dma_start(
                    out=t2q_d[q][:, :].rearrange("(k p) n -> p k n", p=128),
                    in_=t_sb2[:, 2 * q:2 * q + 2, :],
                )
                all_gather(t2q_d[q], t2g_d[q])

            # ======= s2 = Winv @ t2_full, slot-pipelined single pass =======
            ps2 = [
                psq.tile([128, 2, 2, C], F32, tag="ps", name=f"ps2_{i}")
                for i in range(2)
            ]

            def s2_mm(p, rhs):
                for mt in range(8):
                    nc.tensor.matmul(
                        ps2[mt // 4][:, (mt % 4) // 2, (mt % 4) % 2, :],
                        winv_sb[:, p, mt * 128:(mt + 1) * 128],
                        rhs,
                        start=(p == 0 and mt % 2 == 0),
                        stop=(p == NSLOT - 1),
                        skip_group_check=True,
                    )

            for p in range(8):
                s2_mm(p, t_sb2[:, p, :])
            for r in range(28):
                qq, s = divmod(r, NREM)
                tsg = tsp.tile([128, 2, C], BF16, tag="ts", name=f"ts2_{r}")
                nc.scalar.dma_start(
                    out=tsg[:],
                    in_=t2g_d[qq][bass.ts(rrs[s], Q), :].rearrange(
                        "(k p) n -> p k n", p=128
                    ),
                )
                for k in range(2):
                    s2_mm(8 + 2 * r + k, tsg[:, k, :])

            for q in range(4):
                for j in range(2):
                    mt = 2 * q + j
                    nc.vector.tensor_scalar_mul(
                        s_sb2[:, mt, :],
                        ps2[mt // 4][:, (mt % 4) // 2, (mt % 4) % 2, :],
                        f2_sb[:, mt:mt + 1],
                    )
                nc.scalar.dma_start(
                    out=s2q_d[q][:, :].rearrange("(k p) n -> p k n", p=128),
                    in_=s_sb2[:, 2 * q:2 * q + 2, :],
                )
                all_gather(s2q_d[q], s2g_d[q])

            # ================= layer 2 out =================
            # out_st reuses a "wt" slot; allocated lazily AFTER o2's last wT
            # tile so the ring rotation never makes a wT load wait on the
            # final output stores.
            _oh = {}

            def out_drain(ch, po):
                if "t" not in _oh:
                    _oh["t"] = wtp.tile([128, 2, R], F32, tag="wt", name="out_st")
                out_st = _oh["t"]
                for mh in range(2):
                    nc.vector.tensor_copy(
                        out_st[:, ch, mh * 512:(mh + 1) * 512],
                        po[:, mh * 512:(mh + 1) * 512],
                    )
                    nc.scalar.dma_start(
                        out=outT.ap()[
                            ch * 128:(ch + 1) * 128, mh * 512:(mh + 1) * 512
                        ],
                        in_=out_st[:, ch, mh * 512:(mh + 1) * 512],
                    )

            o_phase(s_sb2, s2g_d, out_drain, "o2")

    nc.compile()
    return nc


_NC_CACHE = {}


def _get_nc():
    if "nc" not in _NC_CACHE:
        _NC_CACHE["nc"] = build_kernel()
    return _NC_CACHE["nc"]


def _perm(i):
    """Contraction-block order for core i: own 1024 rows, then quarter-major /
    cyclic-rank remote quarters (AG arrival order)."""
    idx = [np.arange(i * R, (i + 1) * R)]
    for q in range(4):
        for s in range(NREM):
            rr = (i + 1 + s) % NCORES
            base = rr * R + q * Q
            idx.append(np.arange(base, base + Q))
    return np.concatenate(idx)


def make_in_maps(input, wavelets, wavelets_inv, W1, W2, filter1, filter2):
    input = np.asarray(input, np.float32)
    wavelets = np.asarray(wavelets, np.float32)
    wavelets_inv = np.asarray(wavelets_inv, np.float32)
    W1b = np.ascontiguousarray(np.asarray(W1, np.float32)).astype(NP_BF16)
    W2b = np.ascontiguousarray(np.asarray(W2, np.float32)).astype(NP_BF16)
    filter1 = np.asarray(filter1, np.float32)
    filter2 = np.asarray(filter2, np.float32)

    xTf = np.ascontiguousarray(input.T).astype(NP_BF16)   # [F, N]
    in_maps = []
    for i in range(NCORES):
        r0, r1 = i * R, (i + 1) * R
        perm = _perm(i)
        winvT_i = np.ascontiguousarray(wavelets_inv[r0:r1].T).astype(NP_BF16)
        wT_i = np.ascontiguousarray(wavelets[r0:r1].T).astype(NP_BF16)
        in_maps.append(
            {
                "xT": xTf,
                "w1": W1b,
                "w2": W2b,
                "winvT": np.ascontiguousarray(winvT_i[perm]),
                "wT": np.ascontiguousarray(wT_i[perm]),
                "f1": np.ascontiguousarray(filter1[r0:r1]),
                "f2": np.ascontiguousarray(filter2[r0:r1]),
            }
        )
    return in_maps


def run(in_maps, trace=False, **kw):
    nc = _get_nc()
    return bass_utils.run_bass_kernel_spmd(
        nc, in_maps, core_ids=list(range(NCORES)), trace=trace, **kw
    )


def kernel(input, wavelets, wavelets_inv, W1, W2, filter1, filter2):
    in_maps = make_in_maps(
        input, wavelets, wavelets_inv, W1, W2, filter1, filter2
    )
    res = run(in_maps)
    out = np.empty((N, C), np.float32)
    for i in range(NCORES):
        out[i * R:(i + 1) * R, :] = res.results[i]["outT"].T
    return out
